# revision 19
# baseline (speedup 1.0000x reference)
"""MHA forward kernel for Trainium2 (Bass/Tile), sharded over (batch, head)
pairs across 8 NeuronCores.

Math (per (b,h) pair):
    scores = softmax(Q K^T / sqrt(64) + bias)   # bias broadcast over (b,h)
    out    = scores @ V

v3 design (vs. v2):
  * bias handled as exp(s + b) = exp(s) * exp(b): exp(biasT) is precomputed
    on the HOST in bf16; the device multiplies it in on the DVE (bf16
    tensor_tensor, 2x mode) -- no PE ident-matmuls, no DVE STT bias adds.
  * the 1/sqrt(64) scale rides the ACT activation's free `scale` operand.
  * V is extended with a ones-column ([V | 1], M=65), so MM2 lands the
    softmax denominator on PSUM partition 64; reciprocal on the host.
  * exps are issued in big PSUM chunks (3,3,3,3,2,2 k-tiles) from two
    3-bank score tiles; ACT (the roofline engine at 1 elem/cycle/lane
    @1.2GHz) stays saturated: slot period ~8.3us is ACT+PE co-paced.
  * per-qt phases are software-pipelined one slot deep.
  v3 changes (head/tail focused -- steady state was already engine-bound):
  * Q/K host layout is [PPC, D, S] so a pair loads in ONE DMA each; only
    pairs 0/1 split (qt0-critical chunk first, K before Q tail) so the
    first MM1 starts ~5us earlier.  Pair 0 loads ride the VECTOR queue,
    pair 1 the SCALAR queue (both idle at the head), so issue
    serialization behind the big exp(bias) stream disappears.
  * ~40 fewer DMA descriptors (68 -> ~46): the end-of-kernel semaphore
    teardown storm scales with DMA count.
  * the FINAL slot issues its MM2 k-tiles eagerly (interleaved after its
    exp/mult chunks) instead of as a serial 6us drain after the loop.
  * output DMAs alternate gpsimd/sync queues in the later half so the
    final queue drain isn't serialized behind 16 ring completions.
"""

import os
import sys

import numpy as np

for _p in ("/opt/trn_rl_repo",):
    if _p not in sys.path and os.path.isdir(_p):
        sys.path.insert(0, _p)

B, H, S, D = 2, 16, 2048, 64
N_CORES = 8
PAIRS = B * H                     # 32
PPC = PAIRS // N_CORES            # 4 pairs per core
SCALE = 1.0 / 8.0                 # 1/sqrt(64), applied inside ACT

KT = S // 128                     # 16 k-tiles of 128
QTILE = 512
QT = S // QTILE                   # 4 q-tiles
# exp/mult chunks as (start_kt, end_kt); alternate between the two 3-bank
# score tiles A,B,A,B,A,B -- the EVEN chunk count is load-bearing: slot t
# ends on B and slot t+1 starts on A, so the A-tile WAR never gates the
# ACT stream across slot boundaries
CHUNKS = [(0, 3), (3, 6), (6, 9), (9, 12), (12, 14), (14, 16)]

_CACHE = {}


def _chunk_of(kt):
    for ci, (a, b) in enumerate(CHUNKS):
        if a <= kt < b:
            return ci
    raise ValueError(kt)


def _build_nc():
    import concourse.mybir as mybir
    import concourse.tile as tile
    from concourse import bacc

    f32 = mybir.dt.float32
    bf16 = mybir.dt.bfloat16
    nc = bacc.Bacc(None)

    # Q^T/K^T carry a DUPLICATE of the 64 head-dims on partitions 64-127:
    # alternate MM1 k-tiles run on PE row-tiles T0/T8 (tile_position rows 0
    # and 64), so consecutive MM1s co-execute instead of merely pipelining.
    qT = nc.declare_dram_parameter("qT", [PPC, 128, S], bf16, isOutput=False)
    kT = nc.declare_dram_parameter("kT", [PPC, 128, S], bf16, isOutput=False)
    v2 = nc.declare_dram_parameter(
        "v2", [PPC, 128, KT, D + 1], bf16, isOutput=False
    )
    ebp = nc.declare_dram_parameter(
        "ebp", [QT, 128, KT, QTILE], bf16, isOutput=False
    )
    outU = nc.declare_dram_parameter("outU", [PPC, D + 1, S], f32, isOutput=True)

    with tile.TileContext(nc) as tc:
        with (
            tc.tile_pool(name="eb", bufs=1) as eb_pool,
            tc.tile_pool(name="qk", bufs=4) as qk_pool,
            tc.tile_pool(name="vv", bufs=4) as v_pool,
            tc.tile_pool(name="pp", bufs=2) as p_pool,
            tc.tile_pool(name="epi", bufs=2) as epi_pool,
            tc.tile_pool(name="scA", bufs=1, space="PSUM") as scA_pool,
            tc.tile_pool(name="scB", bufs=1, space="PSUM") as scB_pool,
            tc.tile_pool(name="acc", bufs=2, space="PSUM") as acc_pool,
        ):
            # Later pairs load as three single DMAs on the (otherwise idle)
            # GpSimd queue, well ahead of use.
            def load_pair(p):
                qd = qk_pool.tile([128, S], bf16, tag="q")
                kd = qk_pool.tile([128, S], bf16, tag="k")
                nc.gpsimd.dma_start(kd[:], kT[p])
                nc.gpsimd.dma_start(qd[:], qT[p])
                v_sb = v_pool.tile([128, KT, D + 1], bf16, tag="v")
                nc.gpsimd.dma_start(v_sb[:], v2[p])
                return qd, kd, v_sb

            # Pairs 0/1: smallest critical pieces first -- slot (p,qt0)'s
            # MM1 needs Q's first 512 cols plus ALL of K (K columns are the
            # contraction k; every k-tile is touched).  K's qt0 chunk, then
            # Q's, then the K tail, Q tail, V.  Pair 0 rides the Scalar
            # queue (idle until ~10us), pair 1 the GpSimd queue, so the
            # critical loads issue in parallel and stay off the Sync queue
            # where the 8MB exp(bias) stream would delay them.
            def load_pair_head(p, e):
                qd = qk_pool.tile([128, S], bf16, tag="q", name=f"qd{p}")
                kd = qk_pool.tile([128, S], bf16, tag="k", name=f"kd{p}")
                e.dma_start(kd[:, 0:QTILE], kT[p, :, 0:QTILE])
                e.dma_start(qd[:, 0:QTILE], qT[p, :, 0:QTILE])
                e.dma_start(kd[:, QTILE:S], kT[p, :, QTILE:S])
                e.dma_start(qd[:, QTILE:S], qT[p, :, QTILE:S])
                v_sb = v_pool.tile([128, KT, D + 1], bf16, tag="v", name=f"v{p}")
                e.dma_start(v_sb[:], v2[p])
                return qd, kd, v_sb

            loaded = {
                0: load_pair_head(0, nc.scalar),
                1: load_pair_head(1, nc.gpsimd),
            }

            # warm the ACT exp table before the first real chunk
            dummy = epi_pool.tile([1, 8], f32, tag="dummy")
            nc.vector.memset(dummy[:], 0.0)
            nc.scalar.activation(
                dummy[:], dummy[:], mybir.ActivationFunctionType.Exp
            )
            # warm the PE clock (HAM) with a burst of small matmuls during
            # the initial load wait: sustained PE activity flips the clock
            # gate to 8/8 before real MM1 work begins
            warm_w = epi_pool.tile([128, 64], bf16, tag="warmw")
            nc.vector.memset(warm_w[:], 0.0)
            # 68 back-to-back (they pipeline at ~53ns apiece): PE activity
            # stays continuous from ~7.2us until the first real MM1's
            # operands land (~12us), giving HAM the longest possible
            # uninterrupted busy window to trigger the 8/8 clock early
            warm_o = acc_pool.tile([D + 1, QTILE], f32, tag="osum", name="warm_o")
            for _ in range(68):
                nc.tensor.matmul(
                    warm_o[0:64, 0:64], warm_w[:, 0:64], warm_w[:],
                    start=True, stop=True,
                )

            # exp(bias)^T resident in SBUF: [128, KT, S], qt-major DMA order
            # on the Sync queue (pair-interleaved slots double each
            # qt-slice's arrival deadline, so one queue keeps up).  qt0 is
            # split in 4 (its first chunk gates slot 0's DVE multiply);
            # qt1-3 load whole -- fewer DMA descriptors means a shorter
            # end-of-kernel semaphore teardown.
            eb_sb = eb_pool.tile([128, KT, S], bf16)
            for k4 in range(0, KT, 4):
                nc.sync.dma_start(
                    eb_sb[:, k4 : k4 + 4, 0:QTILE], ebp[0, :, k4 : k4 + 4, :]
                )
            for qc in range(1, QT):
                nc.sync.dma_start(
                    eb_sb[:, :, qc * QTILE : (qc + 1) * QTILE], ebp[qc]
                )

            def mm1(p, qt, kt, sc_tiles):
                qd, kd, _ = loaded[p]
                ci = _chunk_of(kt)
                slot = kt - CHUNKS[ci][0]
                # alternate row-tiles T0/T8 (operand base partition 0/64)
                # so consecutive k-tiles co-execute on the PE
                off = 64 * (kt % 2)
                nc.tensor.matmul(
                    sc_tiles[ci][:, slot, :],
                    kd[off : off + D, kt * 128 : (kt + 1) * 128],
                    qd[off : off + D, qt * QTILE : (qt + 1) * QTILE],
                    start=True,
                    stop=True,
                )

            def exp_mult(p, qt, ci, sc_tiles, p_sb):
                a, b = CHUNKS[ci]
                n = b - a
                sc = sc_tiles[ci]
                nc.scalar.activation(
                    p_sb[:, a:b, :],
                    sc[:, 0:n, :],
                    mybir.ActivationFunctionType.Exp,
                    scale=SCALE,
                )
                nc.vector.tensor_mul(
                    p_sb[:, a:b, :],
                    p_sb[:, a:b, :],
                    eb_sb[:, a:b, qt * QTILE : (qt + 1) * QTILE],
                )

            def mm2(prev, kts):
                p, qt, p_sb, o_psum = prev
                _, _, v_sb = loaded[p]
                for kt in kts:
                    nc.tensor.matmul(
                        o_psum[:],
                        v_sb[:, kt, :],
                        p_sb[:, kt, :],
                        start=(kt == 0),
                        stop=(kt == KT - 1),
                    )

            # raw numerator + sums rows out to DRAM; host divides.  Two
            # consecutive q-tiles of a pair share one [65, 1024] staging
            # tile and go out as ONE DMA (halves the descriptor count).
            # Later-half slots alternate onto the Sync queue (eb stream is
            # done by then) so neither queue's final drain waits on a long
            # ring of completions.
            osb2 = {}

            def epilogue(prev, si):
                p, qt, p_sb, o_psum = prev
                half = qt % 2
                if half == 0:
                    osb2[p] = epi_pool.tile(
                        [D + 1, 2 * QTILE], f32, tag="osb", name="osb2"
                    )
                o_sb = osb2[p]
                nc.vector.tensor_copy(
                    o_sb[:, half * QTILE : (half + 1) * QTILE], o_psum[:]
                )
                if p == PPC - 1:
                    # last pair ships each q-tile block immediately: the
                    # kernel-final DMA is 133KB instead of 266KB, so the
                    # terminal queue drain waits on half the wire time
                    q = nc.sync if half == 0 else nc.gpsimd
                    q.dma_start(
                        outU[p, :, qt * QTILE : (qt + 1) * QTILE],
                        o_sb[:, half * QTILE : (half + 1) * QTILE],
                    )
                elif half == 1:
                    q = nc.sync if (si >= 8 and si % 2 == 1) else nc.gpsimd
                    q.dma_start(
                        outU[p, :, (qt - 1) * QTILE : (qt + 1) * QTILE],
                        o_sb[:],
                    )

            # pair-interleaved slot order: each eb qt-slice's arrival
            # deadline doubles, so the single Sync-queue eb stream keeps up
            slots = []
            for blk in range(PPC // 2):
                for q in range(QT):
                    slots.append((2 * blk, q))
                    slots.append((2 * blk + 1, q))
            prev = None
            for si, (p, qt) in enumerate(slots):
                last = si == len(slots) - 1
                if p not in loaded:
                    loaded[p] = load_pair(p)
                # prefetch pairs needed three slots out
                for la in (si + 2, si + 3):
                    if la < len(slots) and slots[la][0] not in loaded:
                        loaded[slots[la][0]] = load_pair(slots[la][0])
                needed = {p} | {slots[la][0] for la in range(si, min(si + 4, len(slots)))}
                if prev is not None:
                    needed.add(prev[0])
                for old in [k for k in loaded if k not in needed]:
                    del loaded[old]

                p_sb = p_pool.tile([128, KT, QTILE], bf16, tag="p")
                o_psum = acc_pool.tile([D + 1, QTILE], f32, tag="osum")
                sc_tiles = {}
                for ci in range(len(CHUNKS)):
                    pool = scA_pool if ci % 2 == 0 else scB_pool
                    sc_tiles[ci] = pool.tile(
                        [128, 3, QTILE], f32, name="sc", tag="sc")

                # exp(ci) issued as soon as its last k-tile lands, with
                # prev's 16 MM2 k-tiles batched in the gaps
                exp_sched = {2: 0, 5: 1, 8: 2, 11: 3, 13: 4, 15: 5}
                mm2_sched = {5: range(0, 3), 8: range(3, 6),
                             11: range(6, 9), 13: range(9, 12),
                             15: range(12, 16)}
                # the FINAL slot has no successor to hide its MM2 in, so it
                # issues its own MM2 k-tiles eagerly once each chunk's
                # exp*eb has had time to complete
                mm2_cur = {8: range(0, 3), 11: range(3, 6),
                           13: range(6, 9), 15: range(9, 12)} if last else {}
                cur = (p, qt, p_sb, o_psum)
                for kt in range(KT):
                    mm1(p, qt, kt, sc_tiles)
                    if kt in exp_sched:
                        exp_mult(p, qt, exp_sched[kt], sc_tiles, p_sb)
                    if prev is not None and kt in mm2_sched:
                        mm2(prev, mm2_sched[kt])
                    if kt in mm2_cur:
                        mm2(cur, mm2_cur[kt])
                if prev is not None:
                    epilogue(prev, si)
                prev = cur
            mm2(prev, range(12, KT))
            epilogue(prev, len(slots))

    return nc


def _get_nc():
    if "nc" not in _CACHE:
        nc = _build_nc()
        nc.finalize()
        _CACHE["nc"] = nc
    return _CACHE["nc"]


def _make_in_maps(mat1, mat2, mat3, bias):
    import ml_dtypes

    bf16 = ml_dtypes.bfloat16
    q = np.asarray(mat1, dtype=np.float32).reshape(PAIRS, S, D)
    k = np.asarray(mat2, dtype=np.float32).reshape(PAIRS, S, D)
    # [PAIRS, 128, S] contiguous transposed layouts, head-dims duplicated
    # on partitions 64-127 (PE row-tile co-execution); one DMA per pair
    qc_ = q.transpose(0, 2, 1).astype(bf16)
    qc_ = np.ascontiguousarray(np.concatenate([qc_, qc_], axis=1))
    kc_ = k.transpose(0, 2, 1).astype(bf16)
    kc_ = np.ascontiguousarray(np.concatenate([kc_, kc_], axis=1))
    v = np.asarray(mat3, dtype=np.float32).reshape(PAIRS, S, D)
    v2 = np.concatenate([v, np.ones((PAIRS, S, 1), np.float32)], axis=2)
    # partition-major [PAIRS, 128, KT, 65] so the device DMA is contiguous
    v2 = v2.reshape(PAIRS, KT, 128, D + 1).transpose(0, 2, 1, 3)
    v2 = np.ascontiguousarray(v2.astype(bf16))
    ebT = np.exp(np.asarray(bias, dtype=np.float32).reshape(S, S).T)
    # [QT, 128, KT, 512] partition-major per q-tile: a whole qt-slice (or
    # any k4 sub-range of it) is one strided DMA
    ebp = ebT.reshape(KT, 128, QT, QTILE).transpose(2, 1, 0, 3)
    ebp = np.ascontiguousarray(ebp.astype(bf16))

    in_maps = []
    for c in range(N_CORES):
        sl = slice(c * PPC, (c + 1) * PPC)
        in_maps.append(
            {
                "qT": qc_[sl],
                "kT": kc_[sl],
                "v2": v2[sl],
                "ebp": ebp,
            }
        )
    return in_maps


def kernel(mat1, mat2, mat3, bias):
    from concourse.bass_utils import run_bass_kernel_spmd

    in_maps = _make_in_maps(mat1, mat2, mat3, bias)
    nc = _get_nc()
    _CACHE["in_maps"] = in_maps
    res = run_bass_kernel_spmd(nc, in_maps, list(range(N_CORES)))
    outs = [res.results[c]["outU"] for c in range(N_CORES)]
    full = np.concatenate(outs, axis=0)          # [PAIRS, D+1, S]
    out = full[:, :D, :] / full[:, D : D + 1, :]
    out = out.transpose(0, 2, 1).reshape(B, H, S, D)
    return np.ascontiguousarray(out.astype(np.float32))
